# revision 12
# baseline (speedup 1.0000x reference)
"""BlurDegradation kernel for 8x TRN2 NeuronCores.

Math: t[b] successive 11x11 depthwise *circular* convolutions compose into a
single circular convolution whose spectrum is the product of the per-step
spectra. The host composes the (tiny) 20 step-kernels into 21 cumulative
spectra with numpy FFTs and selects per-sample spectrum FK[t[b]]; each device
then computes, per image,  out = Re( F* . (FK o (F x F)) . F* ) / N^2  as four
chained dense matmul stages on the PE array (plus DVE/ACT/GpSimd pointwise
work). Contractions always run over the partition dim, so each stage
implicitly transposes and no PE/DMA transposes are needed.

Optimizations vs the naive 4-stage dense chain:
 - Hermitian half-spectrum: y-frequencies k=0..257 only (258 = even, required
   by fp32r); fold weights (1,2,..,2,1,0) and 1/N^2 live in FK (host-side).
 - Stage 2 is matrix-stationary (resident DFT-matrix weights prefetch cleanly)
   with the k-half axis as the cheap *moving* dim (N=258).
 - Stage 3 is data-stationary with *weight-paired* emission: each fresh
   Z-slice weight load is reused by two consecutive matmuls (the reload is
   skipped), halving the fresh-weight-load penalty.
 - Stage 4 is matrix-stationary with natural [y,x] output; the k-half
   contraction uses three full 128-row tiles (0:128, 128:256, 130:258) with
   the double-counted k=130..255 range pre-halved in FK on the host.
 - H / n1 PSUM tiles are staged to SBUF by the Scalar engine (fast PSUM-bank
   release); the pointwise complex multiply is split across Vector + GpSimd.

Sharding: pure data parallel, 8 samples per core, no cross-core comms.
"""

import numpy as np

N = 512
P = 128
T_STEPS = 20
KS = 11
KP = 258            # padded half-spectrum k-dim (even for fp32r)
NCORES = 8
BATCH = 64
CHANNELS = 3
SPC = BATCH // NCORES  # samples per core
IMGS = SPC * CHANNELS  # images per core

USE_F32R = True

_PROGRAM = None
TRACE = False
LAST_EXEC_NS = None
LAST_TRACE = None


def _build_program():
    import concourse.mybir as mybir
    import concourse.tile as tile
    from concourse import bacc

    f32 = mybir.dt.float32
    f32r = mybir.dt.float32r
    mmdt = f32r if USE_F32R else f32

    nc = bacc.Bacc(
        "TRN2", target_bir_lowering=False, debug=False, num_devices=NCORES
    )
    x_d = nc.dram_tensor("x", [IMGS, N, N], mmdt, kind="ExternalInput").ap()
    fkr_d = nc.dram_tensor("fkr", [SPC, N, KP], f32, kind="ExternalInput").ap()
    fki_d = nc.dram_tensor("fki", [SPC, N, KP], f32, kind="ExternalInput").ap()
    mat_names = ["cmat", "snmat"]
    mat_d = {
        nm: nc.dram_tensor(nm, [N, N], mmdt, kind="ExternalInput").ap()
        for nm in mat_names
    }
    # radix-2 half matrices [256, 256]:
    #  s2 (forward, Gauss): even/odd-row slices of C / (-S-C) / (C-S)
    #  s3 (inverse): rows 0:256 x even/odd-column slices of C / S / -S
    h_names = [
        "ec", "ensc", "ecms", "oc", "onsc", "ocms",
        "cme", "sme", "snme", "cmo", "smo", "snmo",
    ]
    hmat_d = {
        nm: nc.dram_tensor(nm, [N // 2, N // 2], mmdt, kind="ExternalInput").ap()
        for nm in h_names
    }
    out_d = nc.dram_tensor("out", [IMGS, N, N], f32, kind="ExternalOutput").ap()

    with tile.TileContext(nc) as tc:
        with (
            tc.tile_pool(name="mats", bufs=1) as mats,
            tc.tile_pool(name="xsp", bufs=2) as xsp,
            tc.tile_pool(name="outp", bufs=2) as outp,
            tc.tile_pool(name="fkp", bufs=2) as fkp,
            tc.tile_pool(name="apool", bufs=2) as apool,
            tc.tile_pool(name="epool", bufs=2) as epool,
            tc.tile_pool(name="zpool", bufs=2) as zpool,
            tc.tile_pool(name="dpool", bufs=2) as dpool,
            tc.tile_pool(name="vpool", bufs=2) as vpool,
            tc.tile_pool(name="pw", bufs=3) as pw,
            tc.tile_pool(name="psum", bufs=8, space="PSUM") as psum,
        ):
            # resident DFT matrices; only cmat/snmat (stage 1) are DMA'd up
            # front — the rest are issued after the first image's x/fk so the
            # PE can start ~20us earlier
            M = {}
            for nm in mat_names:
                M[nm] = mats.tile([P, 4, N], mmdt, name=nm + "_s")
            for nm in ("cmat", "snmat"):
                nc.sync.dma_start(
                    M[nm][:], mat_d[nm].rearrange("(i p) n -> p i n", p=P)
                )
            Cs, Sns = M["cmat"], M["snmat"]
            # radix half-mats, [p, 2, 256] layout
            HM = {}
            for nm in h_names:
                HM[nm] = mats.tile([P, 2, N // 2], mmdt, name=nm + "_s")
            # C/-S rows 130..257, partition-aligned (stage-4 k tail tile)
            Ck2s = mats.tile([P, N], mmdt, name="ck2_s")
            Snk2s = mats.tile([P, N], mmdt, name="snk2_s")

            def issue_deferred_mats():
                for nm in ("ec", "ensc", "ecms", "oc", "onsc", "ocms"):
                    nc.sync.dma_start(
                        HM[nm][:], hmat_d[nm].rearrange("(i p) n -> p i n", p=P)
                    )
                for nm in ("cme", "sme", "snme", "cmo", "smo", "snmo"):
                    nc.sync.dma_start(
                        HM[nm][:], hmat_d[nm].rearrange("(i p) n -> p i n", p=P)
                    )
                nc.sync.dma_start(Ck2s[:], mat_d["cmat"][130:258, :])
                nc.sync.dma_start(Snk2s[:], mat_d["snmat"][130:258, :])

            def emit_st4(Vr, Vi, img):
                # ---- Stage 4 (matrix-stationary, natural orientation):
                # out[y,x] = sum_k C[k,y] Vr[k,x] + (-S)[k,y] Vi[k,x]
                # k tiles: 0:128, 128:256, 130:258 (FK pre-halved on the
                # double-counted 130..255 range)
                # V's free dim is [even x' | odd x'] blocks; the out DMA
                # re-interleaves columns via a strided access pattern
                outs = outp.tile([P, 4, 2, N // 2], f32, tag="outs")
                od = out_d[img].rearrange(
                    "(i p) (j two) -> p i two j", p=P, two=2
                )
                for ym in range(4):
                    ysl = slice(ym * P, (ym + 1) * P)
                    po = psum.tile([P, N], f32, tag="ps", name="po")
                    nc.tensor.matmul(
                        po[:], Cs[:, 0, ysl], Vr[:, 0, :],
                        start=True, stop=False,
                    )
                    nc.tensor.matmul(
                        po[:], Cs[:, 1, ysl], Vr[:, 1, :],
                        start=False, stop=False,
                    )
                    nc.tensor.matmul(
                        po[:], Ck2s[:, ysl], Vr[:, 2, :],
                        start=False, stop=False,
                    )
                    nc.tensor.matmul(
                        po[:], Sns[:, 0, ysl], Vi[:, 0, :],
                        start=False, stop=False,
                    )
                    nc.tensor.matmul(
                        po[:], Sns[:, 1, ysl], Vi[:, 1, :],
                        start=False, stop=False,
                    )
                    nc.tensor.matmul(
                        po[:], Snk2s[:, ysl], Vi[:, 2, :],
                        start=False, stop=True,
                    )
                    nc.any.tensor_copy(
                        out=outs[:, ym, :, :],
                        in_=po[:].rearrange("p (two j) -> p two j", two=2),
                    )
                    nc.sync.dma_start(od[:, ym, 0, :], outs[:, ym, 0, :])
                    nc.sync.dma_start(od[:, ym, 1, :], outs[:, ym, 1, :])

            def load_fk(s):
                # per-sample spectrum, transposed [l, k] layout, k cols 0..257
                fktr = fkp.tile([P, 4, KP], f32, tag="fktr")
                fkti = fkp.tile([P, 4, KP], f32, tag="fkti")
                nc.sync.dma_start(
                    fktr[:], fkr_d[s].rearrange("(i p) n -> p i n", p=P)
                )
                nc.sync.dma_start(
                    fkti[:], fki_d[s].rearrange("(i p) n -> p i n", p=P)
                )
                return fktr, fkti

            def load_x(img):
                xs = xsp.tile([P, 4, N], mmdt, tag="xs")
                nc.sync.dma_start(
                    xs[:], x_d[img].rearrange("(i p) n -> p i n", p=P)
                )
                return xs

            # stage-1 output A is emitted with m (x-coord) rows permuted into
            # even/odd tiles: 0 = {0,2,..,254}, 1 = {256,..,510}, 2 = odd of
            # first half, 3 = odd of second half — radix-2 DIT needs the
            # contraction split by parity, and a strided stationary slice of
            # xs gives it for free.
            M_SLICES = (
                slice(0, 256, 2), slice(256, 512, 2),
                slice(1, 256, 2), slice(257, 512, 2),
            )

            def stage1(xs):
                # ---- Stage 1 (data-stationary, weight-paired):
                # A_r = x^T C[:, :258] ; A_i = x^T (-S)[:, :258]
                Ar = apool.tile([P, 4, KP], mmdt, tag="Ar")
                Ai = apool.tile([P, 4, KP], mmdt, tag="Ai")
                Apb = apool.tile([P, 4, KP], mmdt, tag="Apb")
                for m in range(4):
                    msl = M_SLICES[m]
                    pa = psum.tile([P, N], f32, tag="ps", name="pa")[:, :KP]
                    pb = psum.tile([P, N], f32, tag="ps", name="pb")[:, :KP]
                    for kk in range(4):
                        nc.tensor.matmul(
                            pa[:], xs[:, kk, msl], Cs[:, kk, 0:KP],
                            start=(kk == 0), stop=(kk == 3),
                        )
                        nc.tensor.matmul(
                            pb[:], xs[:, kk, msl], Sns[:, kk, 0:KP],
                            start=(kk == 0), stop=(kk == 3),
                        )
                    nc.scalar.copy(out=Ar[:, m, :], in_=pa[:])
                    nc.scalar.copy(out=Ai[:, m, :], in_=pb[:])
                    nc.gpsimd.tensor_tensor(
                        Apb[:, m, :], Ar[:, m, :], Ai[:, m, :],
                        mybir.AluOpType.add,
                    )
                return Ar, Ai, Apb

            def stage2(fktr, fkti, Ar, Ai, Apb):
                # ---- Stage 2 (radix-2 DIT over m, matrix-stationary Gauss):
                # E = DFT of even-m rows, P = DFT of odd-m rows (twiddle
                # folded into the odd-row matrix slices); outputs span the
                # top half l' = 0..255 only.
                # per branch: m1 = C_b.(Ar+Ai); m2 = (-S-C)_b.Ar;
                #             m3 = (C-S)_b.Ai ; re = m1-m3 ; im = m1+m2
                Er = epool.tile([P, 2, KP], f32, tag="Er")
                Ei = epool.tile([P, 2, KP], f32, tag="Ei")
                Pr = epool.tile([P, 2, KP], f32, tag="Pr")
                Pi = epool.tile([P, 2, KP], f32, tag="Pi")
                BR = ((HM["ec"], HM["ensc"], HM["ecms"], Er, Ei, 0),
                      (HM["oc"], HM["onsc"], HM["ocms"], Pr, Pi, 2))
                for bc, bnsc, bcms, br_re, br_im, a0 in BR:
                    for lt in range(2):
                        lsl = slice(lt * P, (lt + 1) * P)
                        m1 = psum.tile([P, N], f32, tag="ps", name="m1")[:, :KP]
                        m2 = psum.tile([P, N], f32, tag="ps", name="m2")[:, :KP]
                        m3 = psum.tile([P, N], f32, tag="ps", name="m3")[:, :KP]
                        for kt in range(2):
                            ai = a0 + kt
                            nc.tensor.matmul(
                                m1[:], bc[:, kt, lsl], Apb[:, ai, :],
                                start=(kt == 0), stop=(kt == 1),
                            )
                            nc.tensor.matmul(
                                m2[:], bnsc[:, kt, lsl], Ar[:, ai, :],
                                start=(kt == 0), stop=(kt == 1),
                            )
                            nc.tensor.matmul(
                                m3[:], bcms[:, kt, lsl], Ai[:, ai, :],
                                start=(kt == 0), stop=(kt == 1),
                            )
                        m1s = pw.tile([P, KP], f32, tag="m1s")
                        nc.scalar.copy(out=m1s[:], in_=m1[:])
                        nc.vector.tensor_sub(
                            out=br_re[:, lt, :], in0=m1s[:], in1=m3[:]
                        )
                        nc.vector.tensor_add(
                            out=br_im[:, lt, :], in0=m1s[:], in1=m2[:]
                        )
                # butterfly + FK pointwise per l-tile:
                # H[0:256] = E + P ; H[256:512] = E - P ; Z = FK o H
                Ztr = zpool.tile([P, 4, KP], mmdt, tag="Ztr")
                Zti = zpool.tile([P, 4, KP], mmdt, tag="Zti")
                for lm in range(4):
                    lt = lm % 2
                    op = mybir.AluOpType.add if lm < 2 else (
                        mybir.AluOpType.subtract
                    )
                    hrs = pw.tile([P, KP], f32, tag="hrs")
                    his = pw.tile([P, KP], f32, tag="his")
                    nc.gpsimd.tensor_tensor(
                        hrs[:], Er[:, lt, :], Pr[:, lt, :], op
                    )
                    nc.vector.tensor_tensor(
                        out=his[:], in0=Ei[:, lt, :], in1=Pi[:, lt, :], op=op
                    )
                    # pointwise: Ztr = hr o fr - hi o fi
                    #            Zti = hr o fi + hi o fr
                    fr = fktr[:, lm, :]
                    fi = fkti[:, lm, :]
                    tt = pw.tile([P, KP], f32, tag="tt")
                    tu = pw.tile([P, KP], f32, tag="tu")
                    ztr = Ztr[:, lm, :]
                    zti = Zti[:, lm, :]
                    nc.vector.tensor_mul(out=ztr, in0=hrs[:], in1=fr)
                    nc.gpsimd.tensor_tensor(
                        tt[:], his[:], fi, mybir.AluOpType.mult
                    )
                    nc.vector.tensor_sub(out=ztr, in0=ztr, in1=tt[:])
                    nc.gpsimd.tensor_tensor(
                        tu[:], hrs[:], fi, mybir.AluOpType.mult
                    )
                    nc.vector.tensor_mul(out=zti, in0=his[:], in1=fr)
                    nc.vector.tensor_add(out=zti, in0=zti, in1=tu[:])
                return Ztr, Zti

            def stage3(Ztr, Zti):
                # ---- Stage 3 (radix-2 DIF over l, data-stationary):
                # B1 = Ztop + Zbot -> even x' ; D = Ztop - Zbot -> odd x'
                # (inverse twiddle folded into odd-column matrix slices)
                B1r = dpool.tile([P, 2, KP], mmdt, tag="B1r")
                B1i = dpool.tile([P, 2, KP], mmdt, tag="B1i")
                Dr = dpool.tile([P, 2, KP], mmdt, tag="Dr")
                Di = dpool.tile([P, 2, KP], mmdt, tag="Di")
                nc.vector.tensor_sub(
                    out=Dr[:], in0=Ztr[:, 0:2, :], in1=Ztr[:, 2:4, :]
                )
                nc.gpsimd.tensor_tensor(
                    Di[:], Zti[:, 0:2, :], Zti[:, 2:4, :],
                    mybir.AluOpType.subtract,
                )
                nc.vector.tensor_add(
                    out=B1r[:], in0=Ztr[:, 0:2, :], in1=Ztr[:, 2:4, :]
                )
                nc.gpsimd.tensor_tensor(
                    B1i[:], Zti[:, 0:2, :], Zti[:, 2:4, :],
                    mybir.AluOpType.add,
                )
                # V_e = B1^T (Cme + i Sme) ; V_o = D^T (Cmo + i Smo)
                # k M-tiles: 0:128, 128:256, 130:258
                Vr = vpool.tile([P, 3, N], mmdt, tag="Vr")
                Vi = vpool.tile([P, 3, N], mmdt, tag="Vi")
                N2 = N // 2
                BRS = ((B1r, B1i, HM["cme"], HM["sme"], HM["snme"], 0),
                       (Dr, Di, HM["cmo"], HM["smo"], HM["snmo"], N2))
                for km in range(3):
                    koff = (0, 128, 130)[km]
                    ksl = slice(koff, koff + P)
                    for br, bi, cm, sm, snm, xoff in BRS:
                        nvr = psum.tile([P, N], f32, tag="ps", name="nvr")
                        nvr = nvr[:, :N2]
                        nvi = psum.tile([P, N], f32, tag="ps", name="nvi")
                        nvi = nvi[:, :N2]
                        for lt in range(2):
                            nc.tensor.matmul(
                                nvr[:], br[:, lt, ksl], cm[:, lt, :],
                                start=(lt == 0), stop=False,
                            )
                            nc.tensor.matmul(
                                nvi[:], br[:, lt, ksl], sm[:, lt, :],
                                start=(lt == 0), stop=False,
                            )
                        for lt in range(2):
                            nc.tensor.matmul(
                                nvr[:], bi[:, lt, ksl], snm[:, lt, :],
                                start=False, stop=(lt == 1),
                            )
                            nc.tensor.matmul(
                                nvi[:], bi[:, lt, ksl], cm[:, lt, :],
                                start=False, stop=(lt == 1),
                            )
                        nc.any.tensor_copy(
                            out=Vr[:, km, xoff:xoff + N2], in_=nvr[:]
                        )
                        nc.any.tensor_copy(
                            out=Vi[:, km, xoff:xoff + N2], in_=nvi[:]
                        )
                return Vr, Vi

            # Software pipeline: stage 1 of image i+1 is emitted between the
            # previous image's stage 4 and this image's stage 3, filling the
            # PE bubble while stage-2 pointwise (DVE/GpSimd) finishes Z.
            fk_cur = load_fk(0)
            xs_cur = load_x(0)
            issue_deferred_mats()
            a_cur = stage1(xs_cur)
            pending = None
            for img in range(IMGS):
                s = img // CHANNELS
                fk_next, xs_next = fk_cur, None
                if img + 1 < IMGS:
                    s_next = (img + 1) // CHANNELS
                    if s_next != s:
                        fk_next = load_fk(s_next)
                    xs_next = load_x(img + 1)
                Z = stage2(fk_cur[0], fk_cur[1], *a_cur)
                if pending is not None:
                    emit_st4(*pending)
                    pending = None
                if xs_next is not None:
                    a_next = stage1(xs_next)
                else:
                    a_next = None
                V = stage3(*Z)
                pending = (V[0], V[1], img)
                a_cur, fk_cur = a_next, fk_next

            if pending is not None:
                emit_st4(*pending)

    nc.compile()
    return nc


def _host_spectra(kernels):
    """Compose step kernels into 21 cumulative half-spectra, transposed to
    [l, k] layout with Hermitian weights, 1/N^2, and the stage-4
    double-count halving folded in. Returns (FKtr, FKti) f32 [21, 512, KP]."""
    kernels = np.asarray(kernels, dtype=np.float64)
    h = np.zeros((T_STEPS, N, N), np.float64)
    idx = (KS // 2 - np.arange(KS)) % N
    h[:, idx[:, None], idx[None, :]] = kernels
    s_step = np.fft.fft2(h)
    cum = np.ones((T_STEPS + 1, N, N), np.complex128)
    for i in range(1, T_STEPS + 1):
        cum[i] = cum[i - 1] * s_step[i - 1]
    w = np.zeros(KP)
    w[: N // 2 + 1] = 2.0
    w[0] = w[N // 2] = 1.0
    fkt = (cum[:, :KP, :] * w[None, :, None] / float(N * N)).transpose(0, 2, 1)
    half = np.ones(KP)
    half[130:256] = 0.5  # k rows 130..255 appear in both stage-4 k-tiles
    fkt = fkt * half[None, None, :]
    return (
        np.ascontiguousarray(fkt.real.astype(np.float32)),
        np.ascontiguousarray(fkt.imag.astype(np.float32)),
    )


def _dft_mats():
    j = np.arange(N)
    ang = 2.0 * np.pi * (np.outer(j, j) % N) / N
    cm = np.cos(ang).astype(np.float32)
    sm = np.sin(ang).astype(np.float32)
    h = N // 2

    def c(a):
        return np.ascontiguousarray(a)

    # radix-2 s2 (forward DIT, Gauss combos) on even/odd row slices
    ce, se = cm[0::2, :h], sm[0::2, :h]
    co, so = cm[1::2, :h], sm[1::2, :h]
    # radix-2 s3 (inverse DIF) on even/odd column slices
    cme, sme = cm[:h, 0::2], sm[:h, 0::2]
    cmo, smo = cm[:h, 1::2], sm[:h, 1::2]
    return {
        "cmat": cm,
        "snmat": c(-sm),
        "ec": c(ce), "ensc": c(-se - ce), "ecms": c(ce - se),
        "oc": c(co), "onsc": c(-so - co), "ocms": c(co - so),
        "cme": c(cme), "sme": c(sme), "snme": c(-sme),
        "cmo": c(cmo), "smo": c(smo), "snmo": c(-smo),
    }


def kernel(x0, t, kernels):
    global _PROGRAM, LAST_EXEC_NS, LAST_TRACE
    from concourse import bass_utils

    x0 = np.ascontiguousarray(np.asarray(x0), dtype=np.float32)
    tt = np.asarray(t).astype(np.int64)
    fktr_all, fkti_all = _host_spectra(kernels)
    mats = _dft_mats()

    if _PROGRAM is None:
        _PROGRAM = _build_program()
    nc = _PROGRAM

    in_maps = []
    for c in range(NCORES):
        sl = slice(c * SPC, (c + 1) * SPC)
        ts = tt[sl]
        im = {
            "x": np.ascontiguousarray(x0[sl].reshape(IMGS, N, N)),
            "fkr": np.ascontiguousarray(fktr_all[ts]),
            "fki": np.ascontiguousarray(fkti_all[ts]),
        }
        im.update(mats)
        in_maps.append(im)

    res = bass_utils.run_bass_kernel_spmd(
        nc, in_maps, core_ids=list(range(NCORES)), trace=TRACE
    )
    LAST_EXEC_NS = res.exec_time_ns
    if res.instructions_and_trace is not None:
        LAST_TRACE = res.instructions_and_trace[1]
    out = np.empty((BATCH, CHANNELS, N, N), np.float32)
    for c in range(NCORES):
        out[c * SPC : (c + 1) * SPC] = res.results[c]["out"].reshape(
            SPC, CHANNELS, N, N
        )
    return out



# revision 15
# speedup vs baseline: 67.2734x; 67.2734x over previous
"""BlurDegradation kernel for 8x TRN2 NeuronCores.

Math: t[b] successive 11x11 depthwise *circular* convolutions compose into a
single circular convolution whose spectrum is the product of the per-step
spectra. The host composes the (tiny) 20 step-kernels into 21 cumulative
spectra with numpy FFTs and selects per-sample spectrum FK[t[b]]; each device
then computes, per image,  out = Re( F* . (FK o (F x F)) . F* ) / N^2  as four
chained dense matmul stages on the PE array (plus DVE/ACT/GpSimd pointwise
work). Contractions always run over the partition dim, so each stage
implicitly transposes and no PE/DMA transposes are needed.

Optimizations vs the naive 4-stage dense chain:
 - Hermitian half-spectrum: y-frequencies k=0..257 only (258 = even, required
   by fp32r); fold weights (1,2,..,2,1,0) and 1/N^2 live in FK (host-side).
 - Stage 2 is matrix-stationary (resident DFT-matrix weights prefetch cleanly)
   with the k-half axis as the cheap *moving* dim (N=258).
 - Stage 3 is data-stationary with *weight-paired* emission: each fresh
   Z-slice weight load is reused by two consecutive matmuls (the reload is
   skipped), halving the fresh-weight-load penalty.
 - Stage 4 is matrix-stationary with natural [y,x] output; the k-half
   contraction uses three full 128-row tiles (0:128, 128:256, 130:258) with
   the double-counted k=130..255 range pre-halved in FK on the host.
 - H / n1 PSUM tiles are staged to SBUF by the Scalar engine (fast PSUM-bank
   release); the pointwise complex multiply is split across Vector + GpSimd.

Sharding: pure data parallel, 8 samples per core, no cross-core comms.
"""

import numpy as np

N = 512
P = 128
T_STEPS = 20
KS = 11
KP = 258            # padded half-spectrum k-dim (even for fp32r)
NCORES = 8
BATCH = 64
CHANNELS = 3
SPC = BATCH // NCORES  # samples per core
IMGS = SPC * CHANNELS  # images per core

USE_F32R = True

_PROGRAM = None
TRACE = False
LAST_EXEC_NS = None
LAST_TRACE = None


def _build_program():
    import concourse.mybir as mybir
    import concourse.tile as tile
    from concourse import bacc

    f32 = mybir.dt.float32
    f32r = mybir.dt.float32r
    mmdt = f32r if USE_F32R else f32

    nc = bacc.Bacc(
        "TRN2", target_bir_lowering=False, debug=False, num_devices=NCORES
    )
    x_d = nc.dram_tensor("x", [IMGS, N, N], mmdt, kind="ExternalInput").ap()
    fkr_d = nc.dram_tensor("fkr", [SPC, N, KP], f32, kind="ExternalInput").ap()
    fki_d = nc.dram_tensor("fki", [SPC, N, KP], f32, kind="ExternalInput").ap()
    mat_names = ["cmat", "snmat"]
    mat_d = {
        nm: nc.dram_tensor(nm, [N, N], mmdt, kind="ExternalInput").ap()
        for nm in mat_names
    }
    # radix-2 half matrices [256, 256]:
    #  s2 (forward, Gauss): even/odd-row slices of C / (-S-C) / (C-S)
    #  s3 (inverse): rows 0:256 x even/odd-column slices of C / S / -S
    h_names = [
        "ec", "ensc", "ecms", "oc", "onsc", "ocms",
        "cme", "sme", "snme", "cmo", "smo", "snmo",
    ]
    hmat_d = {
        nm: nc.dram_tensor(nm, [N // 2, N // 2], mmdt, kind="ExternalInput").ap()
        for nm in h_names
    }
    out_d = nc.dram_tensor("out", [IMGS, N, N], f32, kind="ExternalOutput").ap()

    with tile.TileContext(nc) as tc:
        with (
            tc.tile_pool(name="mats", bufs=1) as mats,
            tc.tile_pool(name="xsp", bufs=2) as xsp,
            tc.tile_pool(name="outp", bufs=2) as outp,
            tc.tile_pool(name="fkp", bufs=2) as fkp,
            tc.tile_pool(name="apool", bufs=2) as apool,
            tc.tile_pool(name="epool", bufs=2) as epool,
            tc.tile_pool(name="zpool", bufs=2) as zpool,
            tc.tile_pool(name="dpool", bufs=2) as dpool,
            tc.tile_pool(name="vpool", bufs=2) as vpool,
            tc.tile_pool(name="pw", bufs=3) as pw,
            tc.tile_pool(name="psum", bufs=8, space="PSUM") as psum,
        ):
            # resident DFT matrices; only cmat/snmat (stage 1) are DMA'd up
            # front — the rest are issued after the first image's x/fk so the
            # PE can start ~20us earlier
            M = {}
            for nm in mat_names:
                M[nm] = mats.tile([P, 4, N], mmdt, name=nm + "_s")
            for nm in ("cmat", "snmat"):
                nc.sync.dma_start(
                    M[nm][:], mat_d[nm].rearrange("(i p) n -> p i n", p=P)
                )
            Cs, Sns = M["cmat"], M["snmat"]
            # radix half-mats, [p, 2, 256] layout
            HM = {}
            for nm in h_names:
                HM[nm] = mats.tile([P, 2, N // 2], mmdt, name=nm + "_s")
            # C/-S rows 130..257, partition-aligned (stage-4 k tail tile)
            Ck2s = mats.tile([P, N], mmdt, name="ck2_s")
            Snk2s = mats.tile([P, N], mmdt, name="snk2_s")

            def issue_deferred_mats():
                for nm in ("ec", "ensc", "ecms", "oc", "onsc", "ocms"):
                    nc.sync.dma_start(
                        HM[nm][:], hmat_d[nm].rearrange("(i p) n -> p i n", p=P)
                    )
                for nm in ("cme", "sme", "snme", "cmo", "smo", "snmo"):
                    nc.sync.dma_start(
                        HM[nm][:], hmat_d[nm].rearrange("(i p) n -> p i n", p=P)
                    )
                nc.sync.dma_start(Ck2s[:], mat_d["cmat"][130:258, :])
                nc.sync.dma_start(Snk2s[:], mat_d["snmat"][130:258, :])

            def emit_st4(Vr, Vi, img):
                # ---- Stage 4 (matrix-stationary, natural orientation):
                # out[y,x] = sum_k C[k,y] Vr[k,x] + (-S)[k,y] Vi[k,x]
                # k tiles: 0:128, 128:256, 130:258 (FK pre-halved on the
                # double-counted 130..255 range)
                outs = outp.tile([P, 4, N], f32, tag="outs")
                od = out_d[img].rearrange("(i p) n -> p i n", p=P)
                for ym in range(4):
                    ysl = slice(ym * P, (ym + 1) * P)
                    po = psum.tile([P, N], f32, tag="ps", name="po")
                    nc.tensor.matmul(
                        po[:], Cs[:, 0, ysl], Vr[:, 0, :],
                        start=True, stop=False,
                    )
                    nc.tensor.matmul(
                        po[:], Cs[:, 1, ysl], Vr[:, 1, :],
                        start=False, stop=False,
                    )
                    nc.tensor.matmul(
                        po[:], Ck2s[:, ysl], Vr[:, 2, :],
                        start=False, stop=False,
                    )
                    nc.tensor.matmul(
                        po[:], Sns[:, 0, ysl], Vi[:, 0, :],
                        start=False, stop=False,
                    )
                    nc.tensor.matmul(
                        po[:], Sns[:, 1, ysl], Vi[:, 1, :],
                        start=False, stop=False,
                    )
                    nc.tensor.matmul(
                        po[:], Snk2s[:, ysl], Vi[:, 2, :],
                        start=False, stop=True,
                    )
                    nc.any.tensor_copy(out=outs[:, ym, :], in_=po[:])
                    nc.sync.dma_start(od[:, ym, :], outs[:, ym, :])

            def load_fk(s):
                # per-sample spectrum, transposed [l, k] layout, k cols 0..257
                fktr = fkp.tile([P, 4, KP], f32, tag="fktr")
                fkti = fkp.tile([P, 4, KP], f32, tag="fkti")
                nc.sync.dma_start(
                    fktr[:], fkr_d[s].rearrange("(i p) n -> p i n", p=P)
                )
                nc.sync.dma_start(
                    fkti[:], fki_d[s].rearrange("(i p) n -> p i n", p=P)
                )
                return fktr, fkti

            def load_x(img):
                xs = xsp.tile([P, 4, N], mmdt, tag="xs")
                nc.sync.dma_start(
                    xs[:], x_d[img].rearrange("(i p) n -> p i n", p=P)
                )
                return xs

            # stage-1 output A is emitted with m (x-coord) rows permuted into
            # even/odd tiles: 0 = {0,2,..,254}, 1 = {256,..,510}, 2 = odd of
            # first half, 3 = odd of second half — radix-2 DIT needs the
            # contraction split by parity, and a strided stationary slice of
            # xs gives it for free.
            M_SLICES = (
                slice(0, 256, 2), slice(256, 512, 2),
                slice(1, 256, 2), slice(257, 512, 2),
            )

            def stage1(xs):
                # ---- Stage 1 (data-stationary, weight-paired):
                # A_r = x^T C[:, :258] ; A_i = x^T (-S)[:, :258]
                Ar = apool.tile([P, 4, KP], mmdt, tag="Ar")
                Ai = apool.tile([P, 4, KP], mmdt, tag="Ai")
                Apb = apool.tile([P, 4, KP], mmdt, tag="Apb")
                for m in range(4):
                    msl = M_SLICES[m]
                    pa = psum.tile([P, N], f32, tag="ps", name="pa")[:, :KP]
                    pb = psum.tile([P, N], f32, tag="ps", name="pb")[:, :KP]
                    for kk in range(4):
                        nc.tensor.matmul(
                            pa[:], xs[:, kk, msl], Cs[:, kk, 0:KP],
                            start=(kk == 0), stop=(kk == 3),
                        )
                        nc.tensor.matmul(
                            pb[:], xs[:, kk, msl], Sns[:, kk, 0:KP],
                            start=(kk == 0), stop=(kk == 3),
                        )
                    nc.scalar.copy(out=Ar[:, m, :], in_=pa[:])
                    nc.scalar.copy(out=Ai[:, m, :], in_=pb[:])
                    nc.gpsimd.tensor_tensor(
                        Apb[:, m, :], Ar[:, m, :], Ai[:, m, :],
                        mybir.AluOpType.add,
                    )
                return Ar, Ai, Apb

            def stage2(fktr, fkti, Ar, Ai, Apb):
                # ---- Stage 2 (radix-2 DIT over m, matrix-stationary Gauss):
                # E = DFT of even-m rows, P = DFT of odd-m rows (twiddle
                # folded into the odd-row matrix slices); outputs span the
                # top half l' = 0..255 only.
                # per branch: m1 = C_b.(Ar+Ai); m2 = (-S-C)_b.Ar;
                #             m3 = (C-S)_b.Ai ; re = m1-m3 ; im = m1+m2
                Er = epool.tile([P, 2, KP], f32, tag="Er")
                Ei = epool.tile([P, 2, KP], f32, tag="Ei")
                Pr = epool.tile([P, 2, KP], f32, tag="Pr")
                Pi = epool.tile([P, 2, KP], f32, tag="Pi")
                BR = ((HM["ec"], HM["ensc"], HM["ecms"], Er, Ei, 0),
                      (HM["oc"], HM["onsc"], HM["ocms"], Pr, Pi, 2))
                for bc, bnsc, bcms, br_re, br_im, a0 in BR:
                    for lt in range(2):
                        lsl = slice(lt * P, (lt + 1) * P)
                        m1 = psum.tile([P, N], f32, tag="ps", name="m1")[:, :KP]
                        m2 = psum.tile([P, N], f32, tag="ps", name="m2")[:, :KP]
                        m3 = psum.tile([P, N], f32, tag="ps", name="m3")[:, :KP]
                        for kt in range(2):
                            ai = a0 + kt
                            nc.tensor.matmul(
                                m1[:], bc[:, kt, lsl], Apb[:, ai, :],
                                start=(kt == 0), stop=(kt == 1),
                            )
                            nc.tensor.matmul(
                                m2[:], bnsc[:, kt, lsl], Ar[:, ai, :],
                                start=(kt == 0), stop=(kt == 1),
                            )
                            nc.tensor.matmul(
                                m3[:], bcms[:, kt, lsl], Ai[:, ai, :],
                                start=(kt == 0), stop=(kt == 1),
                            )
                        m1s = pw.tile([P, KP], f32, tag="m1s")
                        nc.scalar.copy(out=m1s[:], in_=m1[:])
                        nc.vector.tensor_sub(
                            out=br_re[:, lt, :], in0=m1s[:], in1=m3[:]
                        )
                        nc.vector.tensor_add(
                            out=br_im[:, lt, :], in0=m1s[:], in1=m2[:]
                        )
                # butterfly + FK pointwise per l-tile:
                # H[0:256] = E + P ; H[256:512] = E - P ; Z = FK o H
                Ztr = zpool.tile([P, 4, KP], mmdt, tag="Ztr")
                Zti = zpool.tile([P, 4, KP], mmdt, tag="Zti")
                for lm in range(4):
                    lt = lm % 2
                    op = mybir.AluOpType.add if lm < 2 else (
                        mybir.AluOpType.subtract
                    )
                    hrs = pw.tile([P, KP], f32, tag="hrs")
                    his = pw.tile([P, KP], f32, tag="his")
                    nc.gpsimd.tensor_tensor(
                        hrs[:], Er[:, lt, :], Pr[:, lt, :], op
                    )
                    nc.vector.tensor_tensor(
                        out=his[:], in0=Ei[:, lt, :], in1=Pi[:, lt, :], op=op
                    )
                    # pointwise: Ztr = hr o fr - hi o fi
                    #            Zti = hr o fi + hi o fr
                    fr = fktr[:, lm, :]
                    fi = fkti[:, lm, :]
                    tt = pw.tile([P, KP], f32, tag="tt")
                    tu = pw.tile([P, KP], f32, tag="tu")
                    ztr = Ztr[:, lm, :]
                    zti = Zti[:, lm, :]
                    nc.vector.tensor_mul(out=ztr, in0=hrs[:], in1=fr)
                    nc.gpsimd.tensor_tensor(
                        tt[:], his[:], fi, mybir.AluOpType.mult
                    )
                    nc.vector.tensor_sub(out=ztr, in0=ztr, in1=tt[:])
                    nc.gpsimd.tensor_tensor(
                        tu[:], hrs[:], fi, mybir.AluOpType.mult
                    )
                    nc.vector.tensor_mul(out=zti, in0=his[:], in1=fr)
                    nc.vector.tensor_add(out=zti, in0=zti, in1=tu[:])
                return Ztr, Zti

            def stage3(Ztr, Zti):
                # ---- Stage 3 (radix-2 DIF over l, data-stationary):
                # B1 = Ztop + Zbot -> even x' ; D = Ztop - Zbot -> odd x'
                # (inverse twiddle folded into odd-column matrix slices)
                B1r = dpool.tile([P, 2, KP], mmdt, tag="B1r")
                B1i = dpool.tile([P, 2, KP], mmdt, tag="B1i")
                Dr = dpool.tile([P, 2, KP], mmdt, tag="Dr")
                Di = dpool.tile([P, 2, KP], mmdt, tag="Di")
                nc.vector.tensor_sub(
                    out=Dr[:], in0=Ztr[:, 0:2, :], in1=Ztr[:, 2:4, :]
                )
                nc.gpsimd.tensor_tensor(
                    Di[:], Zti[:, 0:2, :], Zti[:, 2:4, :],
                    mybir.AluOpType.subtract,
                )
                nc.vector.tensor_add(
                    out=B1r[:], in0=Ztr[:, 0:2, :], in1=Ztr[:, 2:4, :]
                )
                nc.gpsimd.tensor_tensor(
                    B1i[:], Zti[:, 0:2, :], Zti[:, 2:4, :],
                    mybir.AluOpType.add,
                )
                # V_e = B1^T (Cme + i Sme) ; V_o = D^T (Cmo + i Smo)
                # k M-tiles: 0:128, 128:256, 130:258
                Vr = vpool.tile([P, 3, N], mmdt, tag="Vr")
                Vi = vpool.tile([P, 3, N], mmdt, tag="Vi")
                N2 = N // 2
                # even/odd x' branches write V interleaved (stride-2 free
                # dim) so stage 4 and the out DMA see the natural layout
                BRS = ((B1r, B1i, HM["cme"], HM["sme"], HM["snme"], 0),
                       (Dr, Di, HM["cmo"], HM["smo"], HM["snmo"], 1))
                for km in range(3):
                    koff = (0, 128, 130)[km]
                    ksl = slice(koff, koff + P)
                    for br, bi, cm, sm, snm, xoff in BRS:
                        nvr = psum.tile([P, N], f32, tag="ps", name="nvr")
                        nvr = nvr[:, :N2]
                        nvi = psum.tile([P, N], f32, tag="ps", name="nvi")
                        nvi = nvi[:, :N2]
                        for lt in range(2):
                            nc.tensor.matmul(
                                nvr[:], br[:, lt, ksl], cm[:, lt, :],
                                start=(lt == 0), stop=False,
                            )
                            nc.tensor.matmul(
                                nvi[:], br[:, lt, ksl], sm[:, lt, :],
                                start=(lt == 0), stop=False,
                            )
                        for lt in range(2):
                            nc.tensor.matmul(
                                nvr[:], bi[:, lt, ksl], snm[:, lt, :],
                                start=False, stop=(lt == 1),
                            )
                            nc.tensor.matmul(
                                nvi[:], bi[:, lt, ksl], cm[:, lt, :],
                                start=False, stop=(lt == 1),
                            )
                        nc.any.tensor_copy(
                            out=Vr[:, km, xoff:N:2], in_=nvr[:]
                        )
                        nc.any.tensor_copy(
                            out=Vi[:, km, xoff:N:2], in_=nvi[:]
                        )
                return Vr, Vi

            # Software pipeline: stage 1 of image i+1 is emitted between the
            # previous image's stage 4 and this image's stage 3, filling the
            # PE bubble while stage-2 pointwise (DVE/GpSimd) finishes Z.
            fk_cur = load_fk(0)
            xs_cur = load_x(0)
            issue_deferred_mats()
            a_cur = stage1(xs_cur)
            pending = None
            for img in range(IMGS):
                s = img // CHANNELS
                fk_next, xs_next = fk_cur, None
                if img + 1 < IMGS:
                    s_next = (img + 1) // CHANNELS
                    if s_next != s:
                        fk_next = load_fk(s_next)
                    xs_next = load_x(img + 1)
                Z = stage2(fk_cur[0], fk_cur[1], *a_cur)
                if pending is not None:
                    emit_st4(*pending)
                    pending = None
                if xs_next is not None:
                    a_next = stage1(xs_next)
                else:
                    a_next = None
                V = stage3(*Z)
                pending = (V[0], V[1], img)
                a_cur, fk_cur = a_next, fk_next

            if pending is not None:
                emit_st4(*pending)

    nc.compile()
    return nc


def _host_spectra(kernels):
    """Compose step kernels into 21 cumulative half-spectra, transposed to
    [l, k] layout with Hermitian weights, 1/N^2, and the stage-4
    double-count halving folded in. Returns (FKtr, FKti) f32 [21, 512, KP]."""
    kernels = np.asarray(kernels, dtype=np.float64)
    h = np.zeros((T_STEPS, N, N), np.float64)
    idx = (KS // 2 - np.arange(KS)) % N
    h[:, idx[:, None], idx[None, :]] = kernels
    s_step = np.fft.fft2(h)
    cum = np.ones((T_STEPS + 1, N, N), np.complex128)
    for i in range(1, T_STEPS + 1):
        cum[i] = cum[i - 1] * s_step[i - 1]
    w = np.zeros(KP)
    w[: N // 2 + 1] = 2.0
    w[0] = w[N // 2] = 1.0
    fkt = (cum[:, :KP, :] * w[None, :, None] / float(N * N)).transpose(0, 2, 1)
    half = np.ones(KP)
    half[130:256] = 0.5  # k rows 130..255 appear in both stage-4 k-tiles
    fkt = fkt * half[None, None, :]
    return (
        np.ascontiguousarray(fkt.real.astype(np.float32)),
        np.ascontiguousarray(fkt.imag.astype(np.float32)),
    )


def _dft_mats():
    j = np.arange(N)
    ang = 2.0 * np.pi * (np.outer(j, j) % N) / N
    cm = np.cos(ang).astype(np.float32)
    sm = np.sin(ang).astype(np.float32)
    h = N // 2

    def c(a):
        return np.ascontiguousarray(a)

    # radix-2 s2 (forward DIT, Gauss combos) on even/odd row slices
    ce, se = cm[0::2, :h], sm[0::2, :h]
    co, so = cm[1::2, :h], sm[1::2, :h]
    # radix-2 s3 (inverse DIF) on even/odd column slices
    cme, sme = cm[:h, 0::2], sm[:h, 0::2]
    cmo, smo = cm[:h, 1::2], sm[:h, 1::2]
    return {
        "cmat": cm,
        "snmat": c(-sm),
        "ec": c(ce), "ensc": c(-se - ce), "ecms": c(ce - se),
        "oc": c(co), "onsc": c(-so - co), "ocms": c(co - so),
        "cme": c(cme), "sme": c(sme), "snme": c(-sme),
        "cmo": c(cmo), "smo": c(smo), "snmo": c(-smo),
    }


def kernel(x0, t, kernels):
    global _PROGRAM, LAST_EXEC_NS, LAST_TRACE
    from concourse import bass_utils

    x0 = np.ascontiguousarray(np.asarray(x0), dtype=np.float32)
    tt = np.asarray(t).astype(np.int64)
    fktr_all, fkti_all = _host_spectra(kernels)
    mats = _dft_mats()

    if _PROGRAM is None:
        _PROGRAM = _build_program()
    nc = _PROGRAM

    in_maps = []
    for c in range(NCORES):
        sl = slice(c * SPC, (c + 1) * SPC)
        ts = tt[sl]
        im = {
            "x": np.ascontiguousarray(x0[sl].reshape(IMGS, N, N)),
            "fkr": np.ascontiguousarray(fktr_all[ts]),
            "fki": np.ascontiguousarray(fkti_all[ts]),
        }
        im.update(mats)
        in_maps.append(im)

    res = bass_utils.run_bass_kernel_spmd(
        nc, in_maps, core_ids=list(range(NCORES)), trace=TRACE
    )
    LAST_EXEC_NS = res.exec_time_ns
    if res.instructions_and_trace is not None:
        LAST_TRACE = res.instructions_and_trace[1]
    out = np.empty((BATCH, CHANNELS, N, N), np.float32)
    for c in range(NCORES):
        out[c * SPC : (c + 1) * SPC] = res.results[c]["out"].reshape(
            SPC, CHANNELS, N, N
        )
    return out



# revision 27
# speedup vs baseline: 109.7333x; 1.6312x over previous
"""BlurDegradation kernel for 8x TRN2 NeuronCores.

Math: t[b] successive 11x11 depthwise *circular* convolutions compose into a
single circular convolution whose spectrum is the product of the per-step
spectra. The host composes the (tiny) 20 step-kernels into 21 cumulative
spectra with numpy FFTs and selects per-sample spectrum FK[t[b]]; each device
then computes, per image,  out = Re( F* . (FK o (F x F)) . F* ) / N^2  as four
chained dense matmul stages on the PE array (plus DVE/ACT/GpSimd pointwise
work). Contractions always run over the partition dim, so each stage
implicitly transposes and no PE/DMA transposes are needed.

Optimizations vs the naive 4-stage dense chain:
 - Hermitian half-spectrum: y-frequencies k=0..257 only (258 = even, required
   by fp32r); fold weights (1,2,..,2,1,0) and 1/N^2 live in FK (host-side).
 - Stage 2 is matrix-stationary (resident DFT-matrix weights prefetch cleanly)
   with the k-half axis as the cheap *moving* dim (N=258).
 - Stage 3 is data-stationary with *weight-paired* emission: each fresh
   Z-slice weight load is reused by two consecutive matmuls (the reload is
   skipped), halving the fresh-weight-load penalty.
 - Stage 4 is matrix-stationary with natural [y,x] output; the k-half
   contraction uses three full 128-row tiles (0:128, 128:256, 130:258) with
   the double-counted k=130..255 range pre-halved in FK on the host.
 - H / n1 PSUM tiles are staged to SBUF by the Scalar engine (fast PSUM-bank
   release); the pointwise complex multiply is split across Vector + GpSimd.

Sharding: pure data parallel, 8 samples per core, no cross-core comms.
"""

import numpy as np

N = 512
P = 128
T_STEPS = 20
KS = 11
KP = 258            # padded half-spectrum k-dim (even for fp32r)
NCORES = 8
BATCH = 64
CHANNELS = 3
SPC = BATCH // NCORES  # samples per core
IMGS = SPC * CHANNELS  # images per core

USE_F32R = True

_PROGRAM = None
TRACE = False
LAST_EXEC_NS = None
LAST_TRACE = None


def _build_program():
    import concourse.mybir as mybir
    import concourse.tile as tile
    from concourse import bacc

    f32 = mybir.dt.float32
    f32r = mybir.dt.float32r
    bf16 = mybir.dt.bfloat16
    mmdt = f32r if USE_F32R else f32

    nc = bacc.Bacc(
        "TRN2", target_bir_lowering=False, debug=False, num_devices=NCORES
    )
    x_d = nc.dram_tensor("x", [IMGS, N, N], mmdt, kind="ExternalInput").ap()
    fkr_d = nc.dram_tensor(
        "fkr", [SPC, N, KP], bf16, kind="ExternalInput"
    ).ap()
    fki_d = nc.dram_tensor(
        "fki", [SPC, N, KP], bf16, kind="ExternalInput"
    ).ap()
    mat_names = ["cmat", "snmat"]
    mat_d = {
        nm: nc.dram_tensor(nm, [N, N], mmdt, kind="ExternalInput").ap()
        for nm in mat_names
    }
    # radix-2 half matrices [256, 256], bf16 (stages 2/3 run bf16):
    #  s2 (forward, Gauss): even/odd-row slices of C / (-S-C) / (C-S)
    #  s3 (inverse): rows 0:256 x even/odd-column slices of C / S / -S
    h_names = [
        "ec", "ensc", "ecms", "oc", "onsc", "ocms",
        "cme", "sme", "snme", "cmo", "smo", "snmo",
    ]
    hmat_d = {
        nm: nc.dram_tensor(nm, [N // 2, N // 2], bf16, kind="ExternalInput").ap()
        for nm in h_names
    }
    out_d = nc.dram_tensor("out", [IMGS, N, N], f32, kind="ExternalOutput").ap()

    with tile.TileContext(nc) as tc:
        with (
            tc.tile_pool(name="mats", bufs=1) as mats,
            tc.tile_pool(name="xsp", bufs=2) as xsp,
            tc.tile_pool(name="outp", bufs=2) as outp,
            tc.tile_pool(name="fkp", bufs=2) as fkp,
            tc.tile_pool(name="apool", bufs=2) as apool,
            tc.tile_pool(name="epool", bufs=2) as epool,
            tc.tile_pool(name="zpool", bufs=2) as zpool,
            tc.tile_pool(name="dpool", bufs=2) as dpool,
            tc.tile_pool(name="vpool", bufs=2) as vpool,
            tc.tile_pool(name="pw", bufs=3) as pw,
            tc.tile_pool(name="psum", bufs=8, space="PSUM") as psum,
        ):
            # resident DFT matrices; only cmat/snmat (stage 1) are DMA'd up
            # front — the rest are issued after the first image's x/fk so the
            # PE can start ~20us earlier
            M = {}
            for nm in mat_names:
                M[nm] = mats.tile([P, 4, N], mmdt, name=nm + "_s")
            for nm in ("cmat", "snmat"):
                nc.sync.dma_start(
                    M[nm][:], mat_d[nm].rearrange("(i p) n -> p i n", p=P)
                )
            Cs, Sns = M["cmat"], M["snmat"]
            # radix half-mats, [p, 2, 256] layout, bf16
            HM = {}
            for nm in h_names:
                HM[nm] = mats.tile([P, 2, N // 2], bf16, name=nm + "_s")
            # C/-S rows 130..257, partition-aligned (stage-4 k tail tile)
            Ck2s = mats.tile([P, N], mmdt, name="ck2_s")
            Snk2s = mats.tile([P, N], mmdt, name="snk2_s")

            def issue_deferred_mats():
                for nm in ("ec", "ensc", "ecms", "oc", "onsc", "ocms"):
                    nc.sync.dma_start(
                        HM[nm][:], hmat_d[nm].rearrange("(i p) n -> p i n", p=P)
                    )
                for nm in ("cme", "sme", "snme", "cmo", "smo", "snmo"):
                    nc.sync.dma_start(
                        HM[nm][:], hmat_d[nm].rearrange("(i p) n -> p i n", p=P)
                    )
                nc.sync.dma_start(Ck2s[:], mat_d["cmat"][130:258, :])
                nc.sync.dma_start(Snk2s[:], mat_d["snmat"][130:258, :])

            def emit_st4(Vr, Vi, img):
                # ---- Stage 4 (matrix-stationary, natural orientation):
                # out[y,x] = sum_k C[k,y] Vr[k,x] + (-S)[k,y] Vi[k,x]
                # k tiles: 0:128, 128:256, 130:258 (FK pre-halved on the
                # double-counted 130..255 range)
                outs = outp.tile([P, 4, N], f32, tag="outs")
                od = out_d[img].rearrange("(i p) n -> p i n", p=P)
                for ym in range(4):
                    ysl = slice(ym * P, (ym + 1) * P)
                    po = psum.tile([P, N], f32, tag="ps", name="po")
                    nc.tensor.matmul(
                        po[:], Cs[:, 0, ysl], Vr[:, 0, :],
                        start=True, stop=False,
                    )
                    nc.tensor.matmul(
                        po[:], Cs[:, 1, ysl], Vr[:, 1, :],
                        start=False, stop=False,
                    )
                    nc.tensor.matmul(
                        po[:], Ck2s[:, ysl], Vr[:, 2, :],
                        start=False, stop=False,
                    )
                    nc.tensor.matmul(
                        po[:], Sns[:, 0, ysl], Vi[:, 0, :],
                        start=False, stop=False,
                    )
                    nc.tensor.matmul(
                        po[:], Sns[:, 1, ysl], Vi[:, 1, :],
                        start=False, stop=False,
                    )
                    nc.tensor.matmul(
                        po[:], Snk2s[:, ysl], Vi[:, 2, :],
                        start=False, stop=True,
                    )
                    nc.scalar.copy(out=outs[:, ym, :], in_=po[:])
                    nc.sync.dma_start(od[:, ym, :], outs[:, ym, :])

            def load_fk(s):
                # per-sample spectrum, transposed [l, k] layout, k cols 0..257
                fktr = fkp.tile([P, 4, KP], bf16, tag="fktr")
                fkti = fkp.tile([P, 4, KP], bf16, tag="fkti")
                nc.sync.dma_start(
                    fktr[:], fkr_d[s].rearrange("(i p) n -> p i n", p=P)
                )
                nc.sync.dma_start(
                    fkti[:], fki_d[s].rearrange("(i p) n -> p i n", p=P)
                )
                return fktr, fkti

            def load_x(img):
                xs = xsp.tile([P, 4, N], mmdt, tag="xs")
                nc.sync.dma_start(
                    xs[:], x_d[img].rearrange("(i p) n -> p i n", p=P)
                )
                return xs

            # stage-1 output A is emitted with m (x-coord) rows permuted into
            # even/odd tiles: 0 = {0,2,..,254}, 1 = {256,..,510}, 2 = odd of
            # first half, 3 = odd of second half — radix-2 DIT needs the
            # contraction split by parity, and a strided stationary slice of
            # xs gives it for free.
            M_SLICES = (
                slice(0, 256, 2), slice(256, 512, 2),
                slice(1, 256, 2), slice(257, 512, 2),
            )

            def stage1(xs):
                # ---- Stage 1 (data-stationary, weight-paired):
                # A_r = x^T C[:, :258] ; A_i = x^T (-S)[:, :258]
                Ar = apool.tile([P, 4, KP], bf16, tag="Ar")
                Ai = apool.tile([P, 4, KP], bf16, tag="Ai")
                Apb = apool.tile([P, 4, KP], bf16, tag="Apb")
                for m in range(4):
                    msl = M_SLICES[m]
                    pa = psum.tile([P, N], f32, tag="ps", name="pa")[:, :KP]
                    pb = psum.tile([P, N], f32, tag="ps", name="pb")[:, :KP]
                    for kk in range(4):
                        nc.tensor.matmul(
                            pa[:], xs[:, kk, msl], Cs[:, kk, 0:KP],
                            start=(kk == 0), stop=(kk == 3),
                        )
                        nc.tensor.matmul(
                            pb[:], xs[:, kk, msl], Sns[:, kk, 0:KP],
                            start=(kk == 0), stop=(kk == 3),
                        )
                    nc.scalar.copy(out=Ar[:, m, :], in_=pa[:])
                    nc.scalar.copy(out=Ai[:, m, :], in_=pb[:])
                    nc.gpsimd.tensor_tensor(
                        Apb[:, m, :], Ar[:, m, :], Ai[:, m, :],
                        mybir.AluOpType.add,
                    )
                return Ar, Ai, Apb

            def stage2(fktr, fkti, Ar, Ai, Apb):
                # ---- Stage 2 (radix-2 DIT over m, matrix-stationary Gauss):
                # E = DFT of even-m rows, P = DFT of odd-m rows (twiddle
                # folded into the odd-row matrix slices); outputs span the
                # top half l' = 0..255 only.
                # per branch: m1 = C_b.(Ar+Ai); m2 = (-S-C)_b.Ar;
                #             m3 = (C-S)_b.Ai ; re = m1-m3 ; im = m1+m2
                Er = epool.tile([P, 2, KP], bf16, tag="Er")
                Ei = epool.tile([P, 2, KP], bf16, tag="Ei")
                Pr = epool.tile([P, 2, KP], bf16, tag="Pr")
                Pi = epool.tile([P, 2, KP], bf16, tag="Pi")
                BR = ((HM["ec"], HM["ensc"], HM["ecms"], Er, Ei, 0),
                      (HM["oc"], HM["onsc"], HM["ocms"], Pr, Pi, 2))
                for bc, bnsc, bcms, br_re, br_im, a0 in BR:
                    for lt in range(2):
                        lsl = slice(lt * P, (lt + 1) * P)
                        m1 = psum.tile([P, N], f32, tag="ps", name="m1")[:, :KP]
                        m2 = psum.tile([P, N], f32, tag="ps", name="m2")[:, :KP]
                        m3 = psum.tile([P, N], f32, tag="ps", name="m3")[:, :KP]
                        for kt in range(2):
                            ai = a0 + kt
                            nc.tensor.matmul(
                                m1[:], bc[:, kt, lsl], Apb[:, ai, :],
                                start=(kt == 0), stop=(kt == 1),
                            )
                            nc.tensor.matmul(
                                m2[:], bnsc[:, kt, lsl], Ar[:, ai, :],
                                start=(kt == 0), stop=(kt == 1),
                            )
                            nc.tensor.matmul(
                                m3[:], bcms[:, kt, lsl], Ai[:, ai, :],
                                start=(kt == 0), stop=(kt == 1),
                            )
                        m1s = pw.tile([P, KP], f32, tag="m1s")
                        nc.scalar.copy(out=m1s[:], in_=m1[:])
                        nc.vector.tensor_sub(
                            out=br_re[:, lt, :], in0=m1s[:], in1=m3[:]
                        )
                        nc.vector.tensor_add(
                            out=br_im[:, lt, :], in0=m1s[:], in1=m2[:]
                        )
                # butterfly + FK pointwise per l-tile:
                # H[0:256] = E + P ; H[256:512] = E - P ; Z = FK o H
                Ztr = zpool.tile([P, 4, KP], bf16, tag="Ztr")
                Zti = zpool.tile([P, 4, KP], bf16, tag="Zti")
                for lm in range(4):
                    lt = lm % 2
                    op = mybir.AluOpType.add if lm < 2 else (
                        mybir.AluOpType.subtract
                    )
                    hrs = pw.tile([P, KP], bf16, tag="hrs")
                    his = pw.tile([P, KP], bf16, tag="his")
                    nc.gpsimd.tensor_tensor(
                        hrs[:], Er[:, lt, :], Pr[:, lt, :], op
                    )
                    nc.vector.tensor_tensor(
                        out=his[:], in0=Ei[:, lt, :], in1=Pi[:, lt, :], op=op
                    )
                    # pointwise: Ztr = hr o fr - hi o fi
                    #            Zti = hr o fi + hi o fr
                    fr = fktr[:, lm, :]
                    fi = fkti[:, lm, :]
                    tt = pw.tile([P, KP], bf16, tag="tt")
                    tu = pw.tile([P, KP], bf16, tag="tu")
                    ztr = Ztr[:, lm, :]
                    zti = Zti[:, lm, :]
                    nc.vector.tensor_mul(out=ztr, in0=hrs[:], in1=fr)
                    nc.gpsimd.tensor_tensor(
                        tt[:], his[:], fi, mybir.AluOpType.mult
                    )
                    nc.vector.tensor_sub(out=ztr, in0=ztr, in1=tt[:])
                    nc.gpsimd.tensor_tensor(
                        tu[:], hrs[:], fi, mybir.AluOpType.mult
                    )
                    nc.vector.tensor_mul(out=zti, in0=his[:], in1=fr)
                    nc.vector.tensor_add(out=zti, in0=zti, in1=tu[:])
                # chain tail: stage-3 butterfly, still on this image's slot
                # B1 = Ztop + Zbot -> even x' ; D = Ztop - Zbot -> odd x'
                B1r = dpool.tile([P, 2, KP], bf16, tag="B1r")
                B1i = dpool.tile([P, 2, KP], bf16, tag="B1i")
                Dr = dpool.tile([P, 2, KP], bf16, tag="Dr")
                Di = dpool.tile([P, 2, KP], bf16, tag="Di")
                nc.vector.tensor_sub(
                    out=Dr[:], in0=Ztr[:, 0:2, :], in1=Ztr[:, 2:4, :]
                )
                nc.gpsimd.tensor_tensor(
                    Di[:], Zti[:, 0:2, :], Zti[:, 2:4, :],
                    mybir.AluOpType.subtract,
                )
                nc.vector.tensor_add(
                    out=B1r[:], in0=Ztr[:, 0:2, :], in1=Ztr[:, 2:4, :]
                )
                nc.gpsimd.tensor_tensor(
                    B1i[:], Zti[:, 0:2, :], Zti[:, 2:4, :],
                    mybir.AluOpType.add,
                )
                return B1r, B1i, Dr, Di

            def stage3(B1r, B1i, Dr, Di):
                # ---- Stage 3 (radix-2 DIF over l, data-stationary):
                # V_e = B1^T (Cme + i Sme) ; V_o = D^T (Cmo + i Smo)
                # (inverse twiddle folded into odd-column matrix slices)
                # k M-tiles: 0:128, 128:256, 130:258
                Vr = vpool.tile([P, 3, N], mmdt, tag="Vr")
                Vi = vpool.tile([P, 3, N], mmdt, tag="Vi")
                N2 = N // 2
                # even/odd x' branches write V interleaved (stride-2 free
                # dim) so stage 4 and the out DMA see the natural layout
                BRS = ((B1r, B1i, HM["cme"], HM["sme"], HM["snme"], 0),
                       (Dr, Di, HM["cmo"], HM["smo"], HM["snmo"], 1))
                for km in range(3):
                    koff = (0, 128, 130)[km]
                    ksl = slice(koff, koff + P)
                    for br, bi, cm, sm, snm, xoff in BRS:
                        nvr = psum.tile([P, N], f32, tag="ps", name="nvr")
                        nvr = nvr[:, :N2]
                        nvi = psum.tile([P, N], f32, tag="ps", name="nvi")
                        nvi = nvi[:, :N2]
                        for lt in range(2):
                            nc.tensor.matmul(
                                nvr[:], br[:, lt, ksl], cm[:, lt, :],
                                start=(lt == 0), stop=False,
                            )
                            nc.tensor.matmul(
                                nvi[:], br[:, lt, ksl], sm[:, lt, :],
                                start=(lt == 0), stop=False,
                            )
                        for lt in range(2):
                            nc.tensor.matmul(
                                nvr[:], bi[:, lt, ksl], snm[:, lt, :],
                                start=False, stop=(lt == 1),
                            )
                            nc.tensor.matmul(
                                nvi[:], bi[:, lt, ksl], cm[:, lt, :],
                                start=False, stop=(lt == 1),
                            )
                        nc.scalar.copy(out=Vr[:, km, xoff:N:2], in_=nvr[:])
                        nc.scalar.copy(out=Vi[:, km, xoff:N:2], in_=nvi[:])
                return Vr, Vi

            # Software pipeline (stage 3 deferred one image): PE order in
            # slot i is s2(i), st4(i-2), s1(i+1), s3(i-1) — the stage-2
            # pointwise chain of image i has a full slot of slack before
            # s3(i) consumes its output.
            fk_cur = load_fk(0)
            xs_cur = load_x(0)
            issue_deferred_mats()
            a_cur = stage1(xs_cur)
            pending_st4 = None
            b_prev = None  # (B1r, B1i, Dr, Di, img) awaiting stage 3
            for img in range(IMGS):
                s = img // CHANNELS
                fk_next, xs_next = fk_cur, None
                if img + 1 < IMGS:
                    s_next = (img + 1) // CHANNELS
                    if s_next != s:
                        fk_next = load_fk(s_next)
                    xs_next = load_x(img + 1)
                B = stage2(fk_cur[0], fk_cur[1], *a_cur)
                if pending_st4 is not None:
                    emit_st4(*pending_st4)
                    pending_st4 = None
                if xs_next is not None:
                    a_next = stage1(xs_next)
                else:
                    a_next = None
                if b_prev is not None:
                    V = stage3(*b_prev[:4])
                    pending_st4 = (V[0], V[1], b_prev[4])
                b_prev = (*B, img)
                a_cur, fk_cur = a_next, fk_next

            # drain: s3 + st4 of the last image (and st4 of the second-last)
            if pending_st4 is not None:
                emit_st4(*pending_st4)
            V = stage3(*b_prev[:4])
            emit_st4(V[0], V[1], b_prev[4])

    nc.compile()
    return nc


def _host_spectra(kernels):
    """Compose step kernels into 21 cumulative half-spectra, transposed to
    [l, k] layout with Hermitian weights, 1/N^2, and the stage-4
    double-count halving folded in. Returns (FKtr, FKti) f32 [21, 512, KP]."""
    kernels = np.asarray(kernels, dtype=np.float64)
    h = np.zeros((T_STEPS, N, N), np.float64)
    idx = (KS // 2 - np.arange(KS)) % N
    h[:, idx[:, None], idx[None, :]] = kernels
    s_step = np.fft.fft2(h)
    cum = np.ones((T_STEPS + 1, N, N), np.complex128)
    for i in range(1, T_STEPS + 1):
        cum[i] = cum[i - 1] * s_step[i - 1]
    w = np.zeros(KP)
    w[: N // 2 + 1] = 2.0
    w[0] = w[N // 2] = 1.0
    fkt = (cum[:, :KP, :] * w[None, :, None] / float(N * N)).transpose(0, 2, 1)
    half = np.ones(KP)
    half[130:256] = 0.5  # k rows 130..255 appear in both stage-4 k-tiles
    fkt = fkt * half[None, None, :]
    return (
        np.ascontiguousarray(fkt.real.astype(np.float32)),
        np.ascontiguousarray(fkt.imag.astype(np.float32)),
    )


def _dft_mats():
    j = np.arange(N)
    ang = 2.0 * np.pi * (np.outer(j, j) % N) / N
    cm = np.cos(ang).astype(np.float32)
    sm = np.sin(ang).astype(np.float32)
    h = N // 2

    import ml_dtypes

    def c(a):
        return np.ascontiguousarray(a.astype(ml_dtypes.bfloat16))

    # radix-2 s2 (forward DIT, Gauss combos) on even/odd row slices
    ce, se = cm[0::2, :h], sm[0::2, :h]
    co, so = cm[1::2, :h], sm[1::2, :h]
    # radix-2 s3 (inverse DIF) on even/odd column slices
    cme, sme = cm[:h, 0::2], sm[:h, 0::2]
    cmo, smo = cm[:h, 1::2], sm[:h, 1::2]
    return {
        "cmat": cm,
        "snmat": np.ascontiguousarray(-sm),
        "ec": c(ce), "ensc": c(-se - ce), "ecms": c(ce - se),
        "oc": c(co), "onsc": c(-so - co), "ocms": c(co - so),
        "cme": c(cme), "sme": c(sme), "snme": c(-sme),
        "cmo": c(cmo), "smo": c(smo), "snmo": c(-smo),
    }


def kernel(x0, t, kernels):
    global _PROGRAM, LAST_EXEC_NS, LAST_TRACE
    from concourse import bass_utils

    x0 = np.ascontiguousarray(np.asarray(x0), dtype=np.float32)
    tt = np.asarray(t).astype(np.int64)
    fktr_all, fkti_all = _host_spectra(kernels)
    mats = _dft_mats()

    if _PROGRAM is None:
        _PROGRAM = _build_program()
    nc = _PROGRAM

    import ml_dtypes

    in_maps = []
    for c in range(NCORES):
        sl = slice(c * SPC, (c + 1) * SPC)
        ts = tt[sl]
        im = {
            "x": np.ascontiguousarray(x0[sl].reshape(IMGS, N, N)),
            "fkr": np.ascontiguousarray(
                fktr_all[ts].astype(ml_dtypes.bfloat16)
            ),
            "fki": np.ascontiguousarray(
                fkti_all[ts].astype(ml_dtypes.bfloat16)
            ),
        }
        im.update(mats)
        in_maps.append(im)

    res = bass_utils.run_bass_kernel_spmd(
        nc, in_maps, core_ids=list(range(NCORES)), trace=TRACE
    )
    LAST_EXEC_NS = res.exec_time_ns
    if res.instructions_and_trace is not None:
        LAST_TRACE = res.instructions_and_trace[1]
    out = np.empty((BATCH, CHANNELS, N, N), np.float32)
    for c in range(NCORES):
        out[c * SPC : (c + 1) * SPC] = res.results[c]["out"].reshape(
            SPC, CHANNELS, N, N
        )
    return out

